# revision 30
# baseline (speedup 1.0000x reference)
"""Multi-head attention (B=4, L=2048, D=1024, H=16) on 8 NeuronCores.

Sharding: core c handles batch b=c//2 and query rows [1024*(c%2), +1024).
The per-core input x is the batch's [2048, 1024] activations ROTATED so the
core's own query rows are rows 0..1023 (softmax over keys is permutation
invariant, so rotating keys+values together is exact). No collectives needed.

All matmul operands are bf16 (same PE rate as f32r here, but half the
SBUF/DMA traffic); accumulation is fp32 PSUM. V (augmented with a per-head
ones column whose PV row becomes the softmax denominator) stays
SBUF-resident. x^T is produced by XBAR transpose-DMAs, not the PE.

The kernel is ONE flat instruction stream: after a short prologue
(x transpose-DMAs + pair-0 projections), the per-pair attention units
(scores -> exp -> PV, lookahead-1) run with the projection chunks for LATER
pairs interleaved between units, so the PE always has independent work
while the Activation engine computes exp (~1.0us/unit vs 0.85us of PE work
per unit). All PSUM evictions ride on DVE/Pool, keeping ACT exp-only. The
output projection for l-half 0 interleaves into the last pair's units.

PSUM budget (8 banks): scores [128,2,512]x2 = 4, PV accum [65,512]x2 = 2,
projections [128,512]x2 = 2; the projection pool is closed before the
output-projection pool (2 banks) opens.
"""

import numpy as np

import sys

for _p in ("/opt/trn_rl_repo", "/opt/pypackages"):
    if _p not in sys.path:
        sys.path.append(_p)

from contextlib import ExitStack

import concourse.bass as bass
import concourse.mybir as mybir
import concourse.tile as tile
from concourse import bacc
from concourse.bass_utils import run_bass_kernel_spmd
from concourse.masks import make_identity

B, L, D, H = 4, 2048, 1024, 16
HD = D // H  # 64
LQ = 1024  # query rows per core
N_CORES = 8
F32 = mybir.dt.float32
BF16 = mybir.dt.bfloat16
AF = mybir.ActivationFunctionType

P = 128
KT_TILES = D // P  # 8 k tiles
ST_TILES = L // P  # 16 s tiles
DT_TILES = D // P  # 8 d tiles = 8 head pairs
LH = 512  # l half width
N_PAIR = H // 2  # 8
SCALE = 1.0 / float(np.sqrt(HD))


def _load_bias(nc, pool, dram, name):
    """[1024] dram vector -> [128, 8] sbuf tile; column t = b[128t:128t+128]."""
    t = pool.tile([P, DT_TILES], F32, name=name)
    nc.gpsimd.dma_start(t[:], dram.rearrange("(t p) -> p t", p=P))
    return t


def build_nc(repeat=1):
    nc = bacc.Bacc(None)

    x_d = nc.declare_dram_parameter("x", [L, D], BF16, isOutput=False)
    wq_d = nc.declare_dram_parameter("wq", [D, D], BF16, isOutput=False)
    wk_d = nc.declare_dram_parameter("wk", [D, D], BF16, isOutput=False)
    wv_d = nc.declare_dram_parameter("wv", [D, D], BF16, isOutput=False)
    wo_d = nc.declare_dram_parameter("wo", [D, D], BF16, isOutput=False)
    bq_d = nc.declare_dram_parameter("bq", [D], F32, isOutput=False)
    bk_d = nc.declare_dram_parameter("bk", [D], F32, isOutput=False)
    bv_d = nc.declare_dram_parameter("bv", [D], F32, isOutput=False)
    bo_d = nc.declare_dram_parameter("bo", [D], F32, isOutput=False)
    y_d = nc.declare_dram_parameter("y", [LQ, D], F32, isOutput=True)

    with tile.TileContext(nc) as tc, ExitStack() as ctx:
      for _rep in range(repeat):
       with ExitStack() as rctx:
        singles = rctx.enter_context(tc.tile_pool(name="singles", bufs=1))
        ident32 = singles.tile([P, P], F32, name="ident32")
        make_identity(nc, ident32[:])
        ident = singles.tile([P, P], BF16, name="ident")
        nc.vector.tensor_copy(ident[:], ident32[:])
        bq_sb = _load_bias(nc, singles, bq_d, "bq")
        bk_sb = _load_bias(nc, singles, bk_d, "bk")
        bv_sb = _load_bias(nc, singles, bv_d, "bv")
        bo_sb = _load_bias(nc, singles, bo_d, "bo")

        # big resident slabs (bf16)
        slab = rctx.enter_context(tc.tile_pool(name="slab", bufs=1))
        qt = slab.tile([P, DT_TILES, LQ], BF16, name="qt")  # [d%128, pair, l]
        kt = slab.tile([P, DT_TILES, L], BF16, name="kt")  # [d%128, pair, s]
        ot = slab.tile([P, DT_TILES, LQ], BF16, name="ot")  # [din%128, dt, l]
        xt = slab.tile([P, KT_TILES, L], BF16, name="xt")  # [k%128, ktile, s]
        # V augmented: [s%128, st, head, 64 vals | 1.0]
        vaug = slab.tile([P, ST_TILES, H, HD + 1], BF16, name="vaug")
        nc.vector.memset(vaug[:, :, :, HD : HD + 1], 1.0)

        wo_sb = slab.tile([P, KT_TILES, D], BF16, name="wo_sb")
        gt_sb = slab.tile([P, DT_TILES, LQ], BF16, name="gt_sb")

        wpool = rctx.enter_context(tc.tile_pool(name="wpool", bufs=6))
        et_pool = rctx.enter_context(tc.tile_pool(name="et", bufs=3))
        otmp_pool = rctx.enter_context(tc.tile_pool(name="otmp", bufs=2))
        rr_pool = rctx.enter_context(tc.tile_pool(name="rr", bufs=2))
        rb_pool = rctx.enter_context(tc.tile_pool(name="rb", bufs=2))

        # ---- weight-tile loads, issued ~one pair-group ahead of use ----
        w_tiles = {}
        wload_queue = [(k, dt) for dt in range(DT_TILES) for k in ("q", "k", "v")]
        w_drams = {"q": wq_d, "k": wk_d, "v": wv_d}

        def issue_next_wload():
            if not wload_queue:
                return
            kind, dt = wload_queue.pop(0)
            t = wpool.tile([P, KT_TILES, P], BF16, name="w_col")
            nc.gpsimd.dma_start(
                t[:],
                w_drams[kind][:, dt * P : (dt + 1) * P].rearrange(
                    "(t p) n -> p t n", p=P
                ),
            )
            w_tiles[(kind, dt)] = t

        # The projection pool lives on the RIGHT side of PSUM so it can be
        # closed mid-stream (the left-side ps_s/ps_o stack stays LIFO) and
        # replaced by the output-projection pool in the same 2 banks.
        proj_stack = ExitStack()
        ps_proj = proj_stack.enter_context(
            tc.tile_pool(name="ps_proj", bufs=2, space="PSUM", side="right")
        )
        proj_pools = {"p": ps_proj}  # swapped to a wider pool in the prologue
        ps_g_holder = {}

        def ensure_ps_g():
            if "g" not in ps_g_holder:
                proj_stack.close()  # frees the 2 projection banks
                ps_g_holder["g"] = rctx.enter_context(
                    tc.tile_pool(name="ps_g", bufs=2, space="PSUM", side="right")
                )
            return ps_g_holder["g"]

        # =================== task emitters ===================
        def proj_qk_chunk(kind, dt, ci):
            """One 512-wide column chunk of the Q or K projection for
            d-tile dt; bias added during the DVE eviction."""
            if ci == 0:
                issue_next_wload()
            w_col = w_tiles[(kind, dt)]
            out_sb, b_sb = (qt, bq_sb) if kind == "q" else (kt, bk_sb)
            ps = proj_pools["p"].tile([P, LH], F32, name="ps_proj")
            for ki in range(KT_TILES):
                nc.tensor.matmul(
                    ps[:],
                    w_col[:, ki, :],
                    xt[:, ki, ci * LH : (ci + 1) * LH],
                    start=(ki == 0),
                    stop=(ki == KT_TILES - 1),
                )
            nc.vector.tensor_scalar_add(
                out_sb[:, dt, ci * LH : (ci + 1) * LH], ps[:], b_sb[:, dt : dt + 1]
            )

        def proj_v_chunk(pair, g):
            """V projection for head pair `pair`, s-tiles 4g..4g+3, staged
            into vaug (cols 0..63 per head; col 64 stays the memset 1.0)."""
            if g == 0:
                issue_next_wload()
            w_col = w_tiles[("v", pair)]
            ps = proj_pools["p"].tile([P, LH], F32, name="ps_proj")
            for sti in range(4):
                st = 4 * g + sti
                for ki in range(KT_TILES):
                    nc.tensor.matmul(
                        ps[:, sti * P : (sti + 1) * P],
                        xt[:, ki, st * P : (st + 1) * P],
                        w_col[:, ki, :],
                        start=(ki == 0),
                        stop=(ki == KT_TILES - 1),
                    )
            nc.vector.tensor_copy(
                vaug[:, 4 * g : 4 * g + 4, 2 * pair : 2 * pair + 2, 0:HD],
                ps[:].rearrange("p (s h d) -> p s h d", s=4, h=2),
            )

        def load_wo():
            nc.gpsimd.dma_start(wo_sb[:], wo_d.rearrange("(t p) n -> p t n", p=P))

        def c_proj_chunk(lt, jg, on_act=False):
            """Output projection y^T chunk: 4 dout-tiles for l-tile lt.
            Evict on DVE while attention still runs; on the (then idle)
            ACT engine in the tail."""
            ps_g_pool = ensure_ps_g()
            ps_g = ps_g_pool.tile([P, 4, P], F32, name="ps_g")
            for jj in range(4):
                j = 4 * jg + jj
                for ki in range(KT_TILES):
                    nc.tensor.matmul(
                        ps_g[:, jj, :],
                        wo_sb[:, ki, j * P : (j + 1) * P],
                        ot[:, ki, lt * P : (lt + 1) * P],
                        start=(ki == 0),
                        stop=(ki == KT_TILES - 1),
                    )
                if on_act:
                    nc.scalar.activation(
                        gt_sb[:, j, lt * P : (lt + 1) * P],
                        ps_g[:, jj, :],
                        AF.Identity,
                        bias=bo_sb[:, j : j + 1],
                    )
                else:
                    nc.vector.tensor_scalar_add(
                        gt_sb[:, j, lt * P : (lt + 1) * P],
                        ps_g[:, jj, :],
                        bo_sb[:, j : j + 1],
                    )

        # =================== B1 unit emitters ===================
        def scores_unit(ps_s_pool, p, lh, st):
            ps_s = ps_s_pool.tile([P, 2, LH], F32, name="ps_s")
            for sub in range(2):
                nc.tensor.matmul(
                    ps_s[:, sub, :],
                    kt[sub * HD : (sub + 1) * HD, p, st * P : (st + 1) * P],
                    qt[sub * HD : (sub + 1) * HD, p, lh * LH : (lh + 1) * LH],
                    start=True,
                    stop=True,
                )
            e2 = et_pool.tile([P, 2, LH], BF16, name="et")
            nc.scalar.activation(e2[:], ps_s[:], AF.Exp, scale=SCALE)
            return e2

        def pv_unit(ps_o, p, lh, st, e2):
            for sub in range(2):
                nc.tensor.matmul(
                    ps_o[sub][:],
                    vaug[:, st, 2 * p + sub, :],
                    e2[:, sub, :],
                    start=(st == 0),
                    stop=(st == ST_TILES - 1),
                )

        def epilogue(ps_o, p, lh):
            """Drain the [65, 512] PV accumulators: row 64 is the softmax
            denominator; normalize rows 0..63, add bv, write ot (bf16)."""
            for sub in range(2):
                o_tmp = otmp_pool.tile([HD + 1, LH], F32, name="o_tmp")
                nc.vector.tensor_copy(o_tmp[:], ps_o[sub][:])
                r_row = rr_pool.tile([1, LH], F32, name="r_row")
                nc.vector.reciprocal(r_row[:], o_tmp[HD : HD + 1, :])
                r_bc = rb_pool.tile([HD, LH], F32, name="r_bc")
                nc.gpsimd.partition_broadcast(r_bc[:], r_row[:])
                dst = ot[sub * HD : (sub + 1) * HD, p, lh * LH : (lh + 1) * LH]
                nc.vector.tensor_mul(dst, o_tmp[0:HD, :], r_bc[:])
                nc.vector.tensor_scalar_add(
                    dst, dst, bv_sb[sub * HD : (sub + 1) * HD, p : p + 1]
                )

        # ============================================================
        # Prologue: XBAR transpose-DMAs bring x^T into xt while pair-0
        # projections are emitted behind them.
        # ============================================================
        issue_next_wload()  # q0
        issue_next_wload()  # k0
        issue_next_wload()  # v0
        # XBAR transpose must land in a contiguous tile (a sliced slab
        # destination produces wrong data on hardware); bounce + DVE copy.
        # s-half 0 lands first; its dependent pair-0 chunks are emitted
        # before the s-half-1 bounce copies so the DVE queue can't delay
        # the first projection evictions. A dedicated prologue PSUM pool
        # (right side, closed before B1) widens projection double-buffering.
        with (
            tc.tile_pool(name="xtb", bufs=3) as xtb_pool,
            tc.tile_pool(name="ps_prol", bufs=3, space="PSUM", side="right") as ps_prol,
        ):
            # bounce copies ride the (otherwise idle) Pool engine so the DVE
            # queue is free for the projection evictions from the start
            for s0, sl in ((0, LQ), (LQ, LQ)):
                for t in range(KT_TILES):
                    eng = (nc.sync, nc.scalar)[t % 2]
                    bt = xtb_pool.tile([P, LQ], BF16, name="xtb")
                    eng.dma_start_transpose(
                        bt[:, 0:sl],
                        x_d[s0 : s0 + sl, t * P : (t + 1) * P],
                    )
                    nc.gpsimd.tensor_copy(
                        xt[:, t, s0 : s0 + sl], bt[:, 0:sl]
                    )
            saved_pool, proj_pools["p"] = proj_pools["p"], ps_prol
            for kind, a, b in (
                ("q", 0, 0), ("q", 0, 1), ("k", 0, 0), ("k", 0, 1),
                ("v", 0, 0), ("v", 0, 1), ("k", 0, 2), ("k", 0, 3),
                ("v", 0, 2), ("v", 0, 3),
            ):
                if kind == "v":
                    proj_v_chunk(a, b)
                else:
                    proj_qk_chunk(kind, a, b)
            proj_pools["p"] = saved_pool

        # ============================================================
        # Main flat pipeline: B1 units with interleaved A/C chunks.
        # ============================================================
        units = [
            (p, lh, st)
            for p in range(N_PAIR)
            for lh in range(2)
            for st in range(ST_TILES)
        ]
        u_of = {u: i for i, u in enumerate(units)}

        a_tasks = []  # (emit_fn, earliest_unit, deadline_unit)
        for p2 in range(1, N_PAIR):
            dl = u_of[(p2, 0, 0)]
            for ci in range(2):
                a_tasks.append((lambda k=p2, c=ci: proj_qk_chunk("q", k, c), 0, dl))
            for ci in range(4):
                a_tasks.append((lambda k=p2, c=ci: proj_qk_chunk("k", k, c), 0, dl))
            for g in range(4):
                a_tasks.append((lambda k=p2, g2=g: proj_v_chunk(k, g2), 0, dl))
        a_tasks.append((load_wo, u_of[(6, 0, 0)], u_of[(7, 0, 0)]))
        for lt in range(4):  # phase C, l-half 0, rides inside pair-7/lh=1
            for jg in range(2):
                a_tasks.append(
                    (
                        lambda l2=lt, j2=jg: c_proj_chunk(l2, j2),
                        u_of[(7, 1, 1)] + 2 * (2 * lt + jg),
                        10**9,
                    )
                )

        bstack = ExitStack()
        ps_s_pool = bstack.enter_context(
            tc.tile_pool(name="ps_s", bufs=2, space="PSUM")
        )
        ps_o_pool = bstack.enter_context(
            tc.tile_pool(name="ps_o", bufs=2, space="PSUM")
        )

        task_state = {"i": 0}

        def drain(i):
            while task_state["i"] < len(a_tasks):
                fn, earliest, deadline = a_tasks[task_state["i"]]
                if deadline <= i or (earliest <= i and 3 * task_state["i"] <= i + 9):
                    fn()
                    task_state["i"] += 1
                else:
                    break

        pair_pso = {}

        def ensure_pso(p, lh):
            if (p, lh) not in pair_pso:
                pair_pso[(p, lh)] = [
                    ps_o_pool.tile([HD + 1, LH], F32, name="ps_o") for _ in range(2)
                ]
            return pair_pso[(p, lh)]

        prev = None
        for i, u in enumerate(units):
            drain(i)
            e2 = scores_unit(ps_s_pool, *u)
            if prev is not None:
                pu, pe = prev
                pv_unit(ensure_pso(pu[0], pu[1]), *pu, pe)
                if pu[2] == ST_TILES - 1:
                    epilogue(pair_pso.pop((pu[0], pu[1])), pu[0], pu[1])
            prev = (u, e2)
        pu, pe = prev
        pv_unit(ensure_pso(pu[0], pu[1]), *pu, pe)
        epilogue(pair_pso.pop((pu[0], pu[1])), pu[0], pu[1])
        while task_state["i"] < len(a_tasks):  # stragglers (C lh0 chunks)
            a_tasks[task_state["i"]][0]()
            task_state["i"] += 1

        bstack.close()  # frees scores + PV PSUM banks

        # ---------------- phase C tail ----------------
        with (
            tc.tile_pool(name="ps_ct", bufs=3, space="PSUM") as ps_ct,
            tc.tile_pool(name="yrow", bufs=2) as y_pool,
        ):

            def emit_y(lt):
                """PE-transpose gt (y^T, bf16) back to [l, dout], evict to
                f32 and DMA out one 128-row slice of y."""
                y_row = y_pool.tile([P, D], F32, name="y_row")
                for a in range(2):
                    pt4 = ps_ct.tile([P, 4, P], BF16, name="pt4_out")
                    for b2 in range(4):
                        j = 4 * a + b2
                        nc.tensor.transpose(
                            pt4[:, b2, :],
                            gt_sb[:, j, lt * P : (lt + 1) * P],
                            ident[:],
                        )
                    nc.vector.tensor_copy(y_row[:, a * LH : (a + 1) * LH], pt4[:])
                    # half-row stores so the last DMAs overlap the transposes
                    nc.sync.dma_start(
                        y_d[lt * P : (lt + 1) * P, a * LH : (a + 1) * LH],
                        y_row[:, a * LH : (a + 1) * LH],
                    )

            for lt in range(4, KT_TILES):  # l-half-1 projections + overlap
                for jg in range(2):
                    c_proj_chunk(lt, jg, on_act=True)
                emit_y(lt - 4)
            for lt in range(4, KT_TILES):
                emit_y(lt)

    nc.finalize()
    return nc


_NC_CACHE = None


def kernel(**inputs):
    global _NC_CACHE
    if _NC_CACHE is None:
        _NC_CACHE = build_nc()
    nc = _NC_CACHE

    import ml_dtypes

    bf16 = ml_dtypes.bfloat16
    q = np.ascontiguousarray(np.asarray(inputs["q"], dtype=np.float32))
    wb = {}
    for k in ("Wq", "Wk", "Wv", "Wo"):
        wb[k] = np.ascontiguousarray(np.asarray(inputs[k]).astype(bf16))
    for k in ("bq", "bk", "bv", "bo"):
        wb[k] = np.ascontiguousarray(np.asarray(inputs[k], dtype=np.float32))

    in_maps = []
    for c in range(N_CORES):
        b, half = c // 2, c % 2
        lo = LQ * half
        x_rot = np.concatenate([q[b, lo:], q[b, :lo]], axis=0).astype(bf16)
        in_maps.append({
            "x": np.ascontiguousarray(x_rot),
            "wq": wb["Wq"], "wk": wb["Wk"], "wv": wb["Wv"], "wo": wb["Wo"],
            "bq": wb["bq"], "bk": wb["bk"], "bv": wb["bv"], "bo": wb["bo"],
        })

    res = run_bass_kernel_spmd(nc, in_maps, core_ids=list(range(N_CORES)))

    out = np.empty((B, L, D), dtype=np.float32)
    for c in range(N_CORES):
        b, half = c // 2, c % 2
        lo = LQ * half
        out[b, lo : lo + LQ, :] = res.results[c]["y"]
    return out
